# revision 3
# baseline (speedup 1.0000x reference)
"""EvidenceNet pairwise-MLP scoring kernel for 8 Trainium2 NeuronCores.

Math (reference):
    img = sign(images_hash)/8, txt = sign(texts_hash)/8          [1024, 64] each
    a[i,k] = (img @ W1[:, :64].T)[i,k] + b1[k]                   [1024, 128]
    t[j,k] = (txt @ W1[:, 64:].T)[j,k]                           [1024, 128]
    negE[i,j] = sum_k W2[0,k] * relu(a[i,k] + t[j,k]) + b2[0]
    posE[i,j] = img[i,:] @ txt[j,:]
    out = [exp(clip(posE/0.5)), exp(clip(negE/0.5))] flattened   [1024*1024, 2]
    (clip at +-15 never binds: |2*negE| < 1, |2*posE| <= 2)

Distribution: data-parallel over image rows; core c owns i in [128c, 128c+128).

Per-core device program (k = the 128 hidden dims lives on partitions):
    tT_h [128k, 1024j]  = W1_txt^T-matmul of sign(txt)      (bf16, SBUF)
    aT   [128k, 128i]   = W1_img^T-matmul of sign(img) + b1 (f32, SBUF)
    for each j-block of 256 and each i:
        r_all[:, i*256:(i+1)*256] = relu(tT_h[:, blk] + aT[:, i])   (DVE/ACT)
    for each j in block:
        psum[:, j] = matmul(lhsT=r_all[:, j::256] (strided, [128k x 128i]),
                            rhs=W2col [128k, 1])           (PE, bf16)
    out[:, odd]  = exp(2*psum + 2*b2)                       (ACT, from PSUM)
    out[:, even] = exp(posE/32), posE = sign-img x sign-txt matmul (exact)
"""
import numpy as np
import ml_dtypes

N_CORES = 8
NI, NT, D, H = 1024, 1024, 64, 128
NI_LOC = NI // N_CORES  # 128
JB = 256                # j-block size
N_BLOCKS = NT // JB     # 4
ACT_EVERY = 4           # every 4th i-row's relu goes to ScalarE (rest on DVE)

_compiled = None  # (nc, input_names) cache


def _build():
    import concourse.bacc as bacc
    import concourse.tile as tile
    import concourse.mybir as mybir

    F32 = mybir.dt.float32
    BF16 = mybir.dt.bfloat16
    AF = mybir.ActivationFunctionType
    ALU = mybir.AluOpType

    nc = bacc.Bacc("TRN2", target_bir_lowering=False, debug=False,
                   num_devices=N_CORES)

    txtT_d = nc.dram_tensor("txtT", [D, NT], F32, kind="ExternalInput").ap()
    imgT_d = nc.dram_tensor("imgT", [D, NI_LOC], F32, kind="ExternalInput").ap()
    w1ti_d = nc.dram_tensor("w1ti", [D, H], BF16, kind="ExternalInput").ap()
    w1tt_d = nc.dram_tensor("w1tt", [D, H], BF16, kind="ExternalInput").ap()
    w2c_d = nc.dram_tensor("w2c", [H, 1], BF16, kind="ExternalInput").ap()
    b1c_d = nc.dram_tensor("b1c", [H, 1], F32, kind="ExternalInput").ap()
    b2s_d = nc.dram_tensor("b2s", [H, 1], F32, kind="ExternalInput").ap()
    out_d = nc.dram_tensor("out", [NI_LOC, 2 * NT], F32, kind="ExternalOutput").ap()

    with tile.TileContext(nc) as tc:
        with tc.tile_pool(name="const", bufs=1) as cpool, \
             tc.tile_pool(name="rall", bufs=2) as rpool, \
             tc.tile_pool(name="outp", bufs=1) as opool, \
             tc.tile_pool(name="ps_s", bufs=2, space="PSUM") as ps_s, \
             tc.tile_pool(name="ps_a", bufs=1, space="PSUM") as ps_a, \
             tc.tile_pool(name="ps_b", bufs=2, space="PSUM") as ps_b:

            # ---- load inputs -------------------------------------------------
            txtT_raw = cpool.tile([D, NT], F32)
            nc.sync.dma_start(txtT_raw[:], txtT_d[:])
            imgT_raw = cpool.tile([D, NI_LOC], F32)
            nc.sync.dma_start(imgT_raw[:], imgT_d[:])
            w1ti = cpool.tile([D, H], BF16)
            nc.sync.dma_start(w1ti[:], w1ti_d[:])
            w1tt = cpool.tile([D, H], BF16)
            nc.sync.dma_start(w1tt[:], w1tt_d[:])
            w2c = cpool.tile([H, 1], BF16)
            nc.sync.dma_start(w2c[:], w2c_d[:])
            b1c = cpool.tile([H, 1], F32)
            nc.sync.dma_start(b1c[:], b1c_d[:])
            b2s = cpool.tile([H, 1], F32)
            nc.sync.dma_start(b2s[:], b2s_d[:])

            # ---- straight-through sign (+-1, bf16-exact) ---------------------
            txtT_s = cpool.tile([D, NT], BF16)
            nc.scalar.activation(txtT_s[:], txtT_raw[:], AF.Sign)
            imgT_s = cpool.tile([D, NI_LOC], BF16)
            nc.scalar.activation(imgT_s[:], imgT_raw[:], AF.Sign)

            # ---- tT_h [128k, 1024j] (bf16) and aT [128k, 128i] (f32) ---------
            tT_h = cpool.tile([H, NT], BF16)
            for hh in range(0, NT, 512):
                ps = ps_s.tile([H, 512], F32, tag="hps")
                nc.tensor.matmul(ps[:], lhsT=w1tt[:], rhs=txtT_s[:, hh:hh + 512],
                                 start=True, stop=True)
                nc.vector.tensor_copy(tT_h[:, hh:hh + 512], ps[:])

            aps = ps_a.tile([H, NI_LOC], F32)
            nc.tensor.matmul(aps[:], lhsT=w1ti[:], rhs=imgT_s[:],
                             start=True, stop=True)
            aT = cpool.tile([H, NI_LOC], F32)
            nc.scalar.activation(aT[:], aps[:], AF.Identity, bias=b1c[:], scale=1.0)

            # ---- posE: exact bf16 matmul of +-1 signs, exp(pos/32) -----------
            out_sb = [opool.tile([NI_LOC, 2 * JB], F32, tag=f"out{b}",
                                 name=f"out_sb{b}")
                      for b in range(N_BLOCKS)]
            for hh in range(0, NT, 512):
                ps = ps_s.tile([NI_LOC, 512], F32, tag="pps")
                nc.tensor.matmul(ps[:], lhsT=imgT_s[:], rhs=txtT_s[:, hh:hh + 512],
                                 start=True, stop=True)
                for q in range(0, 512, JB):
                    b = (hh + q) // JB
                    nc.scalar.activation(out_sb[b][:, 0:2 * JB:2], ps[:, q:q + JB],
                                         AF.Exp, bias=0.0, scale=1.0 / 32.0)

            # ---- main pairwise loop ------------------------------------------
            for b in range(N_BLOCKS):
                jb0 = b * JB
                r_all = rpool.tile([H, NI_LOC * JB], BF16, tag="r")
                for i in range(NI_LOC):
                    dst = r_all[:, i * JB:(i + 1) * JB]
                    src = tT_h[:, jb0:jb0 + JB]
                    if i % ACT_EVERY == ACT_EVERY - 1:
                        nc.scalar.activation(dst, src, AF.Relu,
                                             bias=aT[:, i:i + 1], scale=1.0)
                    else:
                        nc.vector.tensor_scalar(dst, src, aT[:, i:i + 1], 0.0,
                                                op0=ALU.add, op1=ALU.max)
                psb = ps_b.tile([NI_LOC, JB], F32, tag="negps")
                for j in range(JB):
                    nc.tensor.matmul(psb[:, j:j + 1],
                                     lhsT=r_all[:, j:NI_LOC * JB:JB],
                                     rhs=w2c[:], start=True, stop=True)
                nc.scalar.activation(out_sb[b][:, 1:2 * JB:2], psb[:],
                                     AF.Exp, bias=b2s[:], scale=2.0)
                nc.sync.dma_start(out_d[:, 2 * jb0:2 * jb0 + 2 * JB], out_sb[b][:])

    nc.compile()
    return nc


def _get_compiled():
    global _compiled
    if _compiled is None:
        _compiled = _build()
    return _compiled


def run(inputs: dict, trace: bool = False):
    """Shard, run on 8 cores, gather. Returns (full_output, BassKernelResults)."""
    from concourse.bass_utils import run_bass_kernel_spmd

    nc = _get_compiled()

    imgs = np.asarray(inputs["images_hash"], dtype=np.float32)
    txts = np.asarray(inputs["texts_hash"], dtype=np.float32)
    W1 = np.asarray(inputs["W1"], dtype=np.float32)
    b1 = np.asarray(inputs["b1"], dtype=np.float32)
    W2 = np.asarray(inputs["W2"], dtype=np.float32)
    b2 = np.asarray(inputs["b2"], dtype=np.float32)
    task = int(np.asarray(inputs["task_is_i2t"]))

    bf16 = ml_dtypes.bfloat16
    txtT = np.ascontiguousarray(txts.T)                       # [64, 1024]
    w1ti = np.ascontiguousarray(W1[:, :D].T * 0.125).astype(bf16)   # [64, 128]
    w1tt = np.ascontiguousarray(W1[:, D:].T * 0.125).astype(bf16)   # [64, 128]
    w2c = W2.reshape(H, 1).astype(bf16)
    b1c = b1.reshape(H, 1).astype(np.float32)
    b2s = np.full((H, 1), 2.0 * float(b2[0]), dtype=np.float32)

    in_maps = []
    for c in range(N_CORES):
        sl = imgs[c * NI_LOC:(c + 1) * NI_LOC]
        in_maps.append({
            "txtT": txtT,
            "imgT": np.ascontiguousarray(sl.T),
            "w1ti": w1ti, "w1tt": w1tt, "w2c": w2c,
            "b1c": b1c, "b2s": b2s,
        })

    res = run_bass_kernel_spmd(nc, in_maps, list(range(N_CORES)), trace=trace)

    full = np.concatenate(
        [res.results[c]["out"].reshape(-1, 2) for c in range(N_CORES)], axis=0)
    if not task:
        # posE column follows txt-major ordering when task_is_i2t == 0
        full = full.copy()
        full[:, 0] = full[:, 0].reshape(NI, NT).T.reshape(-1)
    return np.ascontiguousarray(full, dtype=np.float32), res


def kernel(**inputs) -> np.ndarray:
    out, _ = run(inputs, trace=False)
    return out


# revision 5
# speedup vs baseline: 2.6908x; 2.6908x over previous
"""EvidenceNet pairwise-MLP scoring kernel for 8 Trainium2 NeuronCores.

Math (reference):
    img = sign(images_hash)/8, txt = sign(texts_hash)/8          [1024, 64] each
    a[i,k] = (img @ W1[:, :64].T)[i,k] + b1[k]                   [1024, 128]
    t[j,k] = (txt @ W1[:, 64:].T)[j,k]                           [1024, 128]
    negE[i,j] = sum_k W2[0,k] * relu(a[i,k] + t[j,k]) + b2[0]
    posE[i,j] = img[i,:] @ txt[j,:]
    out = [exp(clip(posE/0.5)), exp(clip(negE/0.5))] flattened   [1024*1024, 2]
    (clip at +-15 never binds: |2*negE| < 1, |2*posE| <= 2)

Distribution: data-parallel over image rows; core c owns i in [128c, 128c+128).

Per-core device program (k = the 128 hidden dims lives on partitions):
    tT_h [128k, 1024j]  = W1_txt^T-matmul of sign(txt)      (bf16, SBUF)
    aT   [128k, 128i]   = W1_img^T-matmul of sign(img) + b1 (f32, SBUF)
    per i (DVE ~5/8, ACT ~3/8 of rows):
        r_i [128k, 1024j] = relu(tT_h + aT[:, i])           (bf16)
        for jb in 0..8:  # contiguous lhsT, negE lands transposed
            psum_jb[:, i] = matmul(lhsT=r_i[:, jb*128:+128], rhs=W2col)
    negT[jb] = exp(2*psum_jb + 2*b2)  -> out_negT [1024j, 128i]  (ACT)
    out_pos  = exp(posE/32), posE = sign-img x sign-txt matmul (exact bf16)
Host gathers: col0 = pos rows, col1 = negT.T rows, interleave + concat.
"""
import numpy as np
import ml_dtypes

N_CORES = 8
NI, NT, D, H = 1024, 1024, 64, 128
NI_LOC = NI // N_CORES  # 128
NJB = NT // H           # 8 psum column-blocks of 128 j
R_BUFS = 24             # in-flight relu tiles (DVE/ACT run-ahead over PE)

_compiled = None


def _act_rows():
    """Which i-rows ScalarE handles (rest on VectorE). ~3/8, spread evenly."""
    rows = set()
    acc = 0
    for i in range(NI_LOC):
        acc += 3
        if acc >= 8:
            acc -= 8
            rows.add(i)
    return rows


def _build():
    import concourse.bacc as bacc
    import concourse.tile as tile
    import concourse.mybir as mybir

    F32 = mybir.dt.float32
    BF16 = mybir.dt.bfloat16
    AF = mybir.ActivationFunctionType
    ALU = mybir.AluOpType

    nc = bacc.Bacc("TRN2", target_bir_lowering=False, debug=False,
                   num_devices=N_CORES)

    txtT_d = nc.dram_tensor("txtT", [D, NT], F32, kind="ExternalInput").ap()
    imgT_d = nc.dram_tensor("imgT", [D, NI_LOC], F32, kind="ExternalInput").ap()
    w1ti_d = nc.dram_tensor("w1ti", [D, H], BF16, kind="ExternalInput").ap()
    w1tt_d = nc.dram_tensor("w1tt", [D, H], BF16, kind="ExternalInput").ap()
    w2c_d = nc.dram_tensor("w2c", [H, 1], BF16, kind="ExternalInput").ap()
    b1c_d = nc.dram_tensor("b1c", [H, 1], F32, kind="ExternalInput").ap()
    b2s_d = nc.dram_tensor("b2s", [H, 1], F32, kind="ExternalInput").ap()
    pos_d = nc.dram_tensor("pos", [NI_LOC, NT], F32, kind="ExternalOutput").ap()
    negT_d = nc.dram_tensor("negT", [NT, NI_LOC], F32, kind="ExternalOutput").ap()

    act_rows = _act_rows()

    with tile.TileContext(nc) as tc:
        with tc.tile_pool(name="const", bufs=1) as cpool, \
             tc.tile_pool(name="rp", bufs=R_BUFS) as rpool, \
             tc.tile_pool(name="op", bufs=1) as opool:

            # ---- load inputs -------------------------------------------------
            txtT_raw = cpool.tile([D, NT], F32)
            nc.sync.dma_start(txtT_raw[:], txtT_d[:])
            imgT_raw = cpool.tile([D, NI_LOC], F32)
            nc.sync.dma_start(imgT_raw[:], imgT_d[:])
            w1ti = cpool.tile([D, H], BF16)
            nc.sync.dma_start(w1ti[:], w1ti_d[:])
            w1tt = cpool.tile([D, H], BF16)
            nc.sync.dma_start(w1tt[:], w1tt_d[:])
            w2c = cpool.tile([H, 1], BF16)
            nc.sync.dma_start(w2c[:], w2c_d[:])
            b1c = cpool.tile([H, 1], F32)
            nc.sync.dma_start(b1c[:], b1c_d[:])
            b2s = cpool.tile([H, 1], F32)
            nc.sync.dma_start(b2s[:], b2s_d[:])

            # ---- sign (+-1, bf16-exact), h-transforms, posE ------------------
            txtT_s = cpool.tile([D, NT], BF16)
            nc.scalar.activation(txtT_s[:], txtT_raw[:], AF.Sign)
            imgT_s = cpool.tile([D, NI_LOC], BF16)
            nc.scalar.activation(imgT_s[:], imgT_raw[:], AF.Sign)

            tT_h = cpool.tile([H, NT], BF16)
            aT = cpool.tile([H, NI_LOC], F32)
            pos_sb = opool.tile([NI_LOC, NT], F32)

            with tc.tile_pool(name="ps_set", bufs=2, space="PSUM") as ps_s, \
                 tc.tile_pool(name="ps_a", bufs=1, space="PSUM") as ps_a:
                for hh in range(0, NT, 512):
                    ps = ps_s.tile([H, 512], F32, tag="hps")
                    nc.tensor.matmul(ps[:], lhsT=w1tt[:],
                                     rhs=txtT_s[:, hh:hh + 512],
                                     start=True, stop=True)
                    nc.vector.tensor_copy(tT_h[:, hh:hh + 512], ps[:])

                aps = ps_a.tile([H, NI_LOC], F32)
                nc.tensor.matmul(aps[:], lhsT=w1ti[:], rhs=imgT_s[:],
                                 start=True, stop=True)
                nc.scalar.activation(aT[:], aps[:], AF.Identity,
                                     bias=b1c[:], scale=1.0)

                for hh in range(0, NT, 512):
                    ps = ps_s.tile([NI_LOC, 512], F32, tag="pps")
                    nc.tensor.matmul(ps[:], lhsT=imgT_s[:],
                                     rhs=txtT_s[:, hh:hh + 512],
                                     start=True, stop=True)
                    nc.scalar.activation(pos_sb[:, hh:hh + 512], ps[:],
                                         AF.Exp, bias=0.0, scale=1.0 / 32.0)
            nc.sync.dma_start(pos_d[:], pos_sb[:])

            # ---- main pairwise loop (negE transposed: psum[jb] is [128j, 128i])
            with tc.tile_pool(name="ps_m", bufs=1, space="PSUM") as ps_m:
                psums = [ps_m.tile([H, NI_LOC], F32, tag=f"np{jb}",
                                   name=f"negps{jb}")
                         for jb in range(NJB)]
                for i in range(NI_LOC):
                    r = rpool.tile([H, NT], BF16, tag="r")
                    if i in act_rows:
                        nc.scalar.activation(r[:], tT_h[:], AF.Relu,
                                             bias=aT[:, i:i + 1], scale=1.0)
                    else:
                        nc.vector.tensor_scalar(r[:], tT_h[:], aT[:, i:i + 1],
                                                0.0, op0=ALU.add, op1=ALU.max)
                    for jb in range(NJB):
                        nc.tensor.matmul(psums[jb][:, i:i + 1],
                                         lhsT=r[:, jb * H:(jb + 1) * H],
                                         rhs=w2c[:], start=True, stop=True)
                for jb in range(NJB):
                    negT_sb = rpool.tile([H, NI_LOC], F32, tag="negsb",
                                         name=f"negsb{jb}")
                    nc.scalar.activation(negT_sb[:], psums[jb][:],
                                         AF.Exp, bias=b2s[:], scale=2.0)
                    nc.sync.dma_start(negT_d[jb * H:(jb + 1) * H, :], negT_sb[:])

    nc.compile()
    return nc


def _get_compiled():
    global _compiled
    if _compiled is None:
        _compiled = _build()
    return _compiled


def run(inputs: dict, trace: bool = False):
    """Shard, run on 8 cores, gather. Returns (full_output, BassKernelResults)."""
    from concourse.bass_utils import run_bass_kernel_spmd

    nc = _get_compiled()

    imgs = np.asarray(inputs["images_hash"], dtype=np.float32)
    txts = np.asarray(inputs["texts_hash"], dtype=np.float32)
    W1 = np.asarray(inputs["W1"], dtype=np.float32)
    b1 = np.asarray(inputs["b1"], dtype=np.float32)
    W2 = np.asarray(inputs["W2"], dtype=np.float32)
    b2 = np.asarray(inputs["b2"], dtype=np.float32)
    task = int(np.asarray(inputs["task_is_i2t"]))

    bf16 = ml_dtypes.bfloat16
    txtT = np.ascontiguousarray(txts.T)                             # [64, 1024]
    w1ti = np.ascontiguousarray(W1[:, :D].T * 0.125).astype(bf16)   # [64, 128]
    w1tt = np.ascontiguousarray(W1[:, D:].T * 0.125).astype(bf16)   # [64, 128]
    w2c = W2.reshape(H, 1).astype(bf16)
    b1c = b1.reshape(H, 1).astype(np.float32)
    b2s = np.full((H, 1), 2.0 * float(b2[0]), dtype=np.float32)

    in_maps = []
    for c in range(N_CORES):
        sl = imgs[c * NI_LOC:(c + 1) * NI_LOC]
        in_maps.append({
            "txtT": txtT,
            "imgT": np.ascontiguousarray(sl.T),
            "w1ti": w1ti, "w1tt": w1tt, "w2c": w2c,
            "b1c": b1c, "b2s": b2s,
        })

    res = run_bass_kernel_spmd(nc, in_maps, list(range(N_CORES)), trace=trace)

    full = np.empty((NI * NT, 2), dtype=np.float32)
    pos = np.concatenate([res.results[c]["pos"] for c in range(N_CORES)], axis=0)
    neg = np.concatenate([res.results[c]["negT"].T for c in range(N_CORES)],
                         axis=0)
    full[:, 0] = (pos if task else pos.T).reshape(-1)
    full[:, 1] = neg.reshape(-1)
    return full, res


def kernel(**inputs) -> np.ndarray:
    out, _ = run(inputs, trace=False)
    return out


# revision 8
# speedup vs baseline: 3.1468x; 1.1695x over previous
"""EvidenceNet pairwise-MLP scoring kernel for 8 Trainium2 NeuronCores.

Math (reference):
    img = sign(images_hash)/8, txt = sign(texts_hash)/8          [1024, 64] each
    a[i,k] = (img @ W1[:, :64].T)[i,k] + b1[k]                   [1024, 128]
    t[j,k] = (txt @ W1[:, 64:].T)[j,k]                           [1024, 128]
    negE[i,j] = sum_k W2[0,k] * relu(a[i,k] + t[j,k]) + b2[0]
    posE[i,j] = img[i,:] @ txt[j,:]
    out = [exp(clip(posE/0.5)), exp(clip(negE/0.5))] flattened   [1024*1024, 2]
    (clip at +-15 never binds: |2*negE| < 1, |2*posE| <= 2)

Distribution: data-parallel over image rows; core c owns i in [128c, 128c+128).

Per-core device program (k = the 128 hidden dims lives on partitions):
    tT_h [128k, 1024j]  = W1_txt^T-matmul of sign(txt)      (bf16, SBUF)
    aT   [128k, 128i]   = W1_img^T-matmul of sign(img) + b1 (f32, SBUF)
    per i (DVE ~5/8, ACT ~3/8 of rows):
        r_i [128k, 1024j] = relu(tT_h + aT[:, i])           (bf16)
        for jb in 0..8:  # contiguous lhsT, negE lands transposed
            psum_jb[:, i] = matmul(lhsT=r_i[:, jb*128:+128], rhs=W2col)
    negT[jb] = exp(2*psum_jb + 2*b2)  -> out_negT [1024j, 128i]  (ACT)
    out_pos  = exp(posE/32), posE = sign-img x sign-txt matmul (exact bf16)
Host gathers: col0 = pos rows, col1 = negT.T rows, interleave + concat.
"""
import numpy as np
import ml_dtypes

N_CORES = 8
NI, NT, D, H = 1024, 1024, 64, 128
NI_LOC = NI // N_CORES  # 128
NJB = NT // H           # 8 psum column-blocks of 128 j
R_BUFS = 24             # in-flight relu tiles (DVE/ACT run-ahead over PE)

_compiled = None


ACT_NUM, ACT_DEN = 1, 4  # fraction of relu rows on ScalarE (rest on VectorE)


def _act_rows():
    """Which i-rows ScalarE handles (rest on VectorE), spread evenly."""
    rows = set()
    acc = 0
    for i in range(NI_LOC):
        acc += ACT_NUM
        if acc >= ACT_DEN:
            acc -= ACT_DEN
            rows.add(i)
    return rows


def _build():
    import concourse.bacc as bacc
    import concourse.tile as tile
    import concourse.mybir as mybir

    F32 = mybir.dt.float32
    BF16 = mybir.dt.bfloat16
    AF = mybir.ActivationFunctionType
    ALU = mybir.AluOpType

    nc = bacc.Bacc("TRN2", target_bir_lowering=False, debug=False,
                   num_devices=N_CORES)

    txtT_d = nc.dram_tensor("txtT", [D, NT], F32, kind="ExternalInput").ap()
    imgT_d = nc.dram_tensor("imgT", [D, NI_LOC], F32, kind="ExternalInput").ap()
    w1ti_d = nc.dram_tensor("w1ti", [D, H], BF16, kind="ExternalInput").ap()
    w1tt_d = nc.dram_tensor("w1tt", [D, H], BF16, kind="ExternalInput").ap()
    w2c_d = nc.dram_tensor("w2c", [H, 1], BF16, kind="ExternalInput").ap()
    b1c_d = nc.dram_tensor("b1c", [H, 1], F32, kind="ExternalInput").ap()
    b2s_d = nc.dram_tensor("b2s", [H, 1], F32, kind="ExternalInput").ap()
    pos_d = nc.dram_tensor("pos", [NI_LOC, NT], F32, kind="ExternalOutput").ap()
    negT_d = nc.dram_tensor("negT", [NT, NI_LOC], F32, kind="ExternalOutput").ap()

    act_rows = _act_rows()

    with tile.TileContext(nc) as tc:
        with tc.tile_pool(name="const", bufs=1) as cpool, \
             tc.tile_pool(name="rp", bufs=R_BUFS) as rpool, \
             tc.tile_pool(name="op", bufs=1) as opool:

            # ---- trigger the ACT table load at t=0 (no input deps) -----------
            warm = cpool.tile([1, 1], F32)
            nc.vector.memset(warm[:], 0.0)
            nc.scalar.activation(warm[:], warm[:], AF.Exp, bias=0.0, scale=1.0)

            # ---- load inputs (txtT split in halves for pipelining) -----------
            txtT_raw = cpool.tile([D, NT], F32)
            for hh in range(0, NT, 512):
                nc.sync.dma_start(txtT_raw[:, hh:hh + 512],
                                  txtT_d[:, hh:hh + 512])
            imgT_raw = cpool.tile([D, NI_LOC], F32)
            nc.sync.dma_start(imgT_raw[:], imgT_d[:])
            w1ti = cpool.tile([D, H], BF16)
            nc.sync.dma_start(w1ti[:], w1ti_d[:])
            w1tt = cpool.tile([D, H], BF16)
            nc.sync.dma_start(w1tt[:], w1tt_d[:])
            w2c = cpool.tile([H, 1], BF16)
            nc.sync.dma_start(w2c[:], w2c_d[:])
            b1c = cpool.tile([H, 1], F32)
            nc.sync.dma_start(b1c[:], b1c_d[:])
            b2s = cpool.tile([H, 1], F32)
            nc.sync.dma_start(b2s[:], b2s_d[:])

            # ---- sign (+-1, bf16-exact), h-transforms, posE ------------------
            txtT_s = cpool.tile([D, NT], BF16)
            imgT_s = cpool.tile([D, NI_LOC], BF16)
            nc.scalar.activation(imgT_s[:], imgT_raw[:], AF.Sign)
            for hh in range(0, NT, 512):
                nc.scalar.activation(txtT_s[:, hh:hh + 512],
                                     txtT_raw[:, hh:hh + 512], AF.Sign)

            tT_h = cpool.tile([H, NT], BF16)
            aT = cpool.tile([H, NI_LOC], F32)
            pos_sb = opool.tile([NI_LOC, NT], F32)

            with tc.tile_pool(name="ps_set", bufs=2, space="PSUM") as ps_s, \
                 tc.tile_pool(name="ps_a", bufs=1, space="PSUM") as ps_a:
                aps = ps_a.tile([H, NI_LOC], F32)
                nc.tensor.matmul(aps[:], lhsT=w1ti[:], rhs=imgT_s[:],
                                 start=True, stop=True)
                nc.scalar.activation(aT[:], aps[:], AF.Identity,
                                     bias=b1c[:], scale=1.0)

                for hh in range(0, NT, 512):
                    ps = ps_s.tile([H, 512], F32, tag="hps")
                    nc.tensor.matmul(ps[:], lhsT=w1tt[:],
                                     rhs=txtT_s[:, hh:hh + 512],
                                     start=True, stop=True)
                    nc.vector.tensor_copy(tT_h[:, hh:hh + 512], ps[:])

                for hh in range(0, NT, 512):
                    ps = ps_s.tile([NI_LOC, 512], F32, tag="pps")
                    nc.tensor.matmul(ps[:], lhsT=imgT_s[:],
                                     rhs=txtT_s[:, hh:hh + 512],
                                     start=True, stop=True)
                    nc.scalar.activation(pos_sb[:, hh:hh + 512], ps[:],
                                         AF.Exp, bias=0.0, scale=1.0 / 32.0)
            nc.sync.dma_start(pos_d[:], pos_sb[:])

            # ---- main pairwise loop (negE transposed: psum[jb] is [128j, 128i])
            with tc.tile_pool(name="ps_m", bufs=1, space="PSUM") as ps_m:
                psums = [ps_m.tile([H, NI_LOC], F32, tag=f"np{jb}",
                                   name=f"negps{jb}")
                         for jb in range(NJB)]
                negT_sbs = [rpool.tile([H, NI_LOC], F32, tag=f"negsb{jb}",
                                       name=f"negsb{jb}")
                            for jb in range(NJB)]
                IH = NI_LOC // 2
                for half in range(2):
                    for i in range(half * IH, (half + 1) * IH):
                        r = rpool.tile([H, NT], BF16, tag="r")
                        if i in act_rows:
                            nc.scalar.activation(r[:], tT_h[:], AF.Relu,
                                                 bias=aT[:, i:i + 1], scale=1.0)
                        else:
                            nc.vector.tensor_scalar(r[:], tT_h[:],
                                                    aT[:, i:i + 1], 0.0,
                                                    op0=ALU.add, op1=ALU.max)
                        for jb in range(NJB):
                            nc.tensor.matmul(psums[jb][:, i:i + 1],
                                             lhsT=r[:, jb * H:(jb + 1) * H],
                                             rhs=w2c[:], start=True, stop=True)
                    i0 = half * IH
                    for jb in range(NJB):
                        nc.scalar.activation(negT_sbs[jb][:, i0:i0 + IH],
                                             psums[jb][:, i0:i0 + IH],
                                             AF.Exp, bias=b2s[:], scale=2.0)
                        nc.sync.dma_start(negT_d[jb * H:(jb + 1) * H,
                                                 i0:i0 + IH],
                                          negT_sbs[jb][:, i0:i0 + IH])

    nc.compile()
    return nc


def _get_compiled():
    global _compiled
    if _compiled is None:
        _compiled = _build()
    return _compiled


def run(inputs: dict, trace: bool = False):
    """Shard, run on 8 cores, gather. Returns (full_output, BassKernelResults)."""
    from concourse.bass_utils import run_bass_kernel_spmd

    nc = _get_compiled()

    imgs = np.asarray(inputs["images_hash"], dtype=np.float32)
    txts = np.asarray(inputs["texts_hash"], dtype=np.float32)
    W1 = np.asarray(inputs["W1"], dtype=np.float32)
    b1 = np.asarray(inputs["b1"], dtype=np.float32)
    W2 = np.asarray(inputs["W2"], dtype=np.float32)
    b2 = np.asarray(inputs["b2"], dtype=np.float32)
    task = int(np.asarray(inputs["task_is_i2t"]))

    bf16 = ml_dtypes.bfloat16
    txtT = np.ascontiguousarray(txts.T)                             # [64, 1024]
    w1ti = np.ascontiguousarray(W1[:, :D].T * 0.125).astype(bf16)   # [64, 128]
    w1tt = np.ascontiguousarray(W1[:, D:].T * 0.125).astype(bf16)   # [64, 128]
    w2c = W2.reshape(H, 1).astype(bf16)
    b1c = b1.reshape(H, 1).astype(np.float32)
    b2s = np.full((H, 1), 2.0 * float(b2[0]), dtype=np.float32)

    in_maps = []
    for c in range(N_CORES):
        sl = imgs[c * NI_LOC:(c + 1) * NI_LOC]
        in_maps.append({
            "txtT": txtT,
            "imgT": np.ascontiguousarray(sl.T),
            "w1ti": w1ti, "w1tt": w1tt, "w2c": w2c,
            "b1c": b1c, "b2s": b2s,
        })

    res = run_bass_kernel_spmd(nc, in_maps, list(range(N_CORES)), trace=trace)

    full = np.empty((NI * NT, 2), dtype=np.float32)
    pos = np.concatenate([res.results[c]["pos"] for c in range(N_CORES)], axis=0)
    neg = np.concatenate([res.results[c]["negT"].T for c in range(N_CORES)],
                         axis=0)
    full[:, 0] = (pos if task else pos.T).reshape(-1)
    full[:, 1] = neg.reshape(-1)
    return full, res


def kernel(**inputs) -> np.ndarray:
    out, _ = run(inputs, trace=False)
    return out
